# revision 1
# baseline (speedup 1.0000x reference)
"""Bahdanau-attention + reservoir-RNN cell fused Trainium2 kernel.

Data-parallel over batch: B=128 split across 8 NeuronCores (16 rows each).
Weights replicated. Per core, for each batch row b:

    qT = Wa/bias matmuls on PE (q + Ua_b fused into the tanh bias, per o-chunk)
    kT[o,s] = sum_h UaT[h,o] * xT[h,s]        (PE, bf16, psum-accumulated)
    t = tanh(kT + q[b] + Ua_b)                (ACT, psum->sbuf bf16)
    scores  = Va . t  via replicated-Va lhsT  (PE -> scores on all 128 partitions)
    e = exp(scores)   (no max subtraction: |scores| <= sum|Va| ~ 11, safe in f32)
    context = (xT . e) / sum(e)               (DVE bf16 mult + DVE/ACT reduce)
    h_next = tanh([x_t, context] @ WihT + h_prev @ WhhT + bias)   (PE + ACT)

The big contractions run in bf16 (fp32 matmul is 4x slower on PE) with fp32
PSUM accumulation; the small q/h_next matmuls run fp32. Scheduling notes:
 - 40 dummy warmup matmuls keep the PE HAM clock-gate at 8/8 through the
   initial DMA phase.
 - xT[0] is the first DMA on the queue so the first real matmul isn't
   stuck behind weight loads.
 - The input-only h_next terms accumulate into PSUM at b==1 so only the 4
   context-dependent matmuls remain after the main loop.
Measured vs fp32 reference: context ~1.7e-3, h_next ~1.4e-4 absmax-rel.
"""
import numpy as np
import ml_dtypes

import concourse.bacc as bacc
import concourse.tile as tile
from concourse import mybir
from concourse.bass_utils import run_bass_kernel_spmd

BF16 = ml_dtypes.bfloat16

B, S, E, H = 128, 2048, 512, 512
NCORES = 8
BPC = B // NCORES          # batch rows per core
P = 128
HC = H // P                # 4 chunks of 128 along H (and E)
EC = E // P                # 4
RC = EC + HC               # 8 contraction chunks for the rnn input
SC = 4                     # s-chunks
SCW = S // SC              # 512 (one PSUM bank of fp32)

_cache = {}


def _build():
    """Build the per-core Bass program (identical on all 8 cores)."""
    nc = bacc.Bacc("TRN2", target_bir_lowering=False, debug=False)
    f32, bf16 = mybir.dt.float32, mybir.dt.bfloat16

    xT_d = nc.dram_tensor("xT", [BPC, H, S], bf16, kind="ExternalInput")
    uaT_d = nc.dram_tensor("uaT", [H, H], bf16, kind="ExternalInput")
    waT_d = nc.dram_tensor("waT", [H, H], f32, kind="ExternalInput")
    va_d = nc.dram_tensor("va_rep", [HC, P, P], bf16, kind="ExternalInput")
    qbias_d = nc.dram_tensor("qbias", [H], f32, kind="ExternalInput")     # Wa_b + Ua_b
    hpT_d = nc.dram_tensor("hpT", [H, BPC], f32, kind="ExternalInput")    # h_prev shard, transposed
    xtT_d = nc.dram_tensor("xtT", [E, BPC], f32, kind="ExternalInput")    # x_t shard, transposed
    wihT_d = nc.dram_tensor("wihT", [E + H, H], f32, kind="ExternalInput")
    whhT_d = nc.dram_tensor("whhT", [H, H], f32, kind="ExternalInput")
    hbias_d = nc.dram_tensor("hbias", [H], f32, kind="ExternalInput")     # Wih_b + Whh_b

    hn_d = nc.dram_tensor("h_next", [BPC, H], f32, kind="ExternalOutput")
    ctx_d = nc.dram_tensor("context", [BPC, H], f32, kind="ExternalOutput")

    with tile.TileContext(nc) as tc:
        with tc.tile_pool(name="weights", bufs=1) as wp, \
             tc.tile_pool(name="x", bufs=3) as xp, \
             tc.tile_pool(name="t", bufs=2) as tp, \
             tc.tile_pool(name="e", bufs=2) as ep, \
             tc.tile_pool(name="scratch", bufs=1) as scp, \
             tc.tile_pool(name="small", bufs=2) as smp, \
             tc.tile_pool(name="kpsum", bufs=2, space="PSUM") as kpp, \
             tc.tile_pool(name="spsum", bufs=2, space="PSUM") as spp, \
             tc.tile_pool(name="qpsum", bufs=1, space="PSUM") as qpp, \
             tc.tile_pool(name="hnpsum", bufs=1, space="PSUM") as hpp:

            # ---- PE HAM warmup: dummy matmuls overlap the initial DMAs so the
            # clock gate is at 8/8 when the real matmuls start ----
            warm_w = wp.tile([P, P], bf16)
            nc.vector.memset(warm_w[:], 0.125)
            warm_r = wp.tile([P, SCW], bf16)
            nc.vector.memset(warm_r[:], 0.5)
            warm_ps = qpp.tile([P, SCW], f32, tag="qpsum")
            for _ in range(40):
                nc.tensor.matmul(warm_ps[:], warm_w[:], warm_r[:], start=True, stop=True)

            # ---- critical-path DMAs first: b=0's x slab, then the k-matmul
            # weights, then what the q phase needs ----
            xt0 = xp.tile([P, HC, S], bf16, tag="xt")
            nc.sync.dma_start(xt0[:], xT_d[0].rearrange("(c p) s -> p c s", p=P))
            ua_t = wp.tile([P, HC, HC, P], bf16)   # [hp, hc, oc, of]
            nc.sync.dma_start(
                ua_t[:], uaT_d[:].rearrange("(hc hp) (oc of) -> hp hc oc of", hp=P, of=P))
            wa_t = wp.tile([P, HC, HC, P], f32)
            nc.sync.dma_start(
                wa_t[:], waT_d[:].rearrange("(hc hp) (oc of) -> hp hc oc of", hp=P, of=P))
            qbias_row = wp.tile([1, H], f32)
            nc.sync.dma_start(qbias_row[:], qbias_d[:].rearrange("(one n) -> one n", one=1))
            hp_t = wp.tile([P, HC, BPC], f32)
            nc.sync.dma_start(hp_t[:], hpT_d[:].rearrange("(c p) b -> p c b", p=P))
            va_t = wp.tile([P, HC, P], bf16)
            nc.sync.dma_start(va_t[:], va_d[:].rearrange("c p f -> p c f"))
            ones_row = wp.tile([1, BPC], f32)
            nc.vector.memset(ones_row[:], 1.0)

            # ---- q phase: qb[o, b] = q[b, o] + Ua_b[o] (the tanh bias) ----
            qb_t = wp.tile([P, HC, BPC], f32)
            for oc in range(HC):
                qpsum = qpp.tile([P, BPC], f32, tag="qpsum")
                for hc in range(HC):
                    nc.tensor.matmul(qpsum[:], wa_t[:, hc, oc, :], hp_t[:, hc, :],
                                     start=(hc == 0), stop=False)
                nc.tensor.matmul(qpsum[:], qbias_row[0:1, oc * P:(oc + 1) * P],
                                 ones_row[:], start=False, stop=True)
                nc.vector.tensor_copy(qb_t[:, oc, :], qpsum[:])

            # prefetch b=1's x slab ahead of the h_next weight loads
            xt1 = xp.tile([P, HC, S], bf16, tag="xt")
            nc.sync.dma_start(xt1[:], xT_d[1].rearrange("(c p) s -> p c s", p=P))

            # ---- non-critical loads: h_next weights + x_t (needed from b==1) ----
            wih_t = wp.tile([P, RC, H], f32)
            nc.sync.dma_start(wih_t[:], wihT_d[:].rearrange("(c p) n -> p c n", p=P))
            whh_t = wp.tile([P, HC, H], f32)
            nc.sync.dma_start(whh_t[:], whhT_d[:].rearrange("(c p) n -> p c n", p=P))
            hbias_row = wp.tile([1, H], f32)
            nc.sync.dma_start(hbias_row[:], hbias_d[:].rearrange("(one n) -> one n", one=1))
            rnn_t = wp.tile([P, RC, BPC], f32)
            nc.sync.dma_start(rnn_t[:, 0:EC, :], xtT_d[:].rearrange("(c p) b -> p c b", p=P))

            hnpsum = None

            # ---- main loop over batch rows ----
            for b in range(BPC):
                if b == 0:
                    xt_t = xt0
                elif b == 1:
                    xt_t = xt1
                else:
                    xt_t = xp.tile([P, HC, S], bf16, tag="xt")
                    nc.sync.dma_start(xt_t[:], xT_d[b].rearrange("(c p) s -> p c s", p=P))

                if b == 1:
                    # input-only h_next terms; only the 4 context-dependent
                    # matmuls remain for the tail after the main loop
                    hnpsum = hpp.tile([BPC, H], f32)
                    for c in range(EC):
                        nc.tensor.matmul(hnpsum[:], rnn_t[:, c, :], wih_t[:, c, :],
                                         start=(c == 0), stop=False)
                    for hc in range(HC):
                        nc.tensor.matmul(hnpsum[:], hp_t[:, hc, :], whh_t[:, hc, :],
                                         start=False, stop=False)
                    nc.tensor.matmul(hnpsum[:], ones_row[:], hbias_row[:],
                                     start=False, stop=False)

                t_sb = tp.tile([P, HC, S], bf16)
                e_bc = ep.tile([P, S], bf16)
                le_sb = smp.tile([P, SC], f32, tag="le")

                for sc2 in range(SC // 2):       # s super-chunks of 1024
                    for oc in range(HC):
                        kpsum = kpp.tile([P, 2 * SCW], f32)
                        for half in range(2):
                            ssl = slice((2 * sc2 + half) * SCW,
                                        (2 * sc2 + half + 1) * SCW)
                            for hc in range(HC):
                                nc.tensor.matmul(
                                    kpsum[:, half * SCW:(half + 1) * SCW],
                                    ua_t[:, hc, oc, :], xt_t[:, hc, ssl],
                                    start=(hc == 0), stop=(hc == HC - 1))
                        nc.scalar.activation(
                            t_sb[:, oc, 2 * sc2 * SCW:(2 * sc2 + 2) * SCW], kpsum[:],
                            mybir.ActivationFunctionType.Tanh,
                            bias=qb_t[:, oc, b:b + 1], scale=1.0)
                    for half in range(2):
                        sc = 2 * sc2 + half
                        ssl = slice(sc * SCW, (sc + 1) * SCW)
                        spsum = spp.tile([P, SCW], f32)
                        for oc in range(HC):
                            nc.tensor.matmul(spsum[:], va_t[:, oc, :], t_sb[:, oc, ssl],
                                             start=(oc == 0), stop=(oc == HC - 1))
                        nc.scalar.activation(
                            e_bc[:, ssl], spsum[:], mybir.ActivationFunctionType.Exp,
                            accum_out=le_sb[:, sc:sc + 1])

                l1 = smp.tile([P, 1], f32, tag="l1")
                rl = smp.tile([P, 1], f32, tag="rl")
                nc.vector.tensor_reduce(l1[:], le_sb[:], mybir.AxisListType.X,
                                        mybir.AluOpType.add)
                nc.vector.reciprocal(rl[:], l1[:])

                ctx_sb = smp.tile([P, HC], f32, tag="ctx")
                for hc in range(HC):
                    # all-bf16 multiply hits the DVE 2x perf mode; the reduce
                    # accumulates in fp32 (ACT offload measured slower: it sits
                    # on the tanh->scores critical chain)
                    scratch = scp.tile([P, S], bf16)
                    nc.vector.tensor_tensor(scratch[:], xt_t[:, hc, :], e_bc[:],
                                            mybir.AluOpType.mult)
                    nc.vector.tensor_reduce(ctx_sb[:, hc:hc + 1], scratch[:],
                                            mybir.AxisListType.X, mybir.AluOpType.add)
                ctx_f = smp.tile([P, HC], f32, tag="ctxf")
                nc.vector.tensor_scalar_mul(ctx_f[:], ctx_sb[:], rl[:])

                nc.sync.dma_start(ctx_d[b].rearrange("(c p) -> p c", p=P), ctx_f[:])
                nc.vector.tensor_copy(rnn_t[:, EC:RC, b:b + 1], ctx_f[:])

            # ---- h_next tail: the context-dependent matmuls + tanh ----
            for c in range(EC, RC):
                nc.tensor.matmul(hnpsum[:], rnn_t[:, c, :], wih_t[:, c, :],
                                 start=False, stop=(c == RC - 1))
            hn_sb = smp.tile([BPC, H], f32, tag="hn")
            nc.scalar.activation(hn_sb[:], hnpsum[:],
                                 mybir.ActivationFunctionType.Tanh)
            nc.sync.dma_start(hn_d[:], hn_sb[:])

    nc.compile()
    return nc


def _prep_host_inputs(x_t, x_ref_encoded, h_prev, Wa_w, Wa_b, Ua_w, Ua_b,
                      Va_w, Va_b, Wih_w, Wih_b, Whh_w, Whh_b):
    """Shard + transpose + cast on host. Returns in_maps for the 8 cores."""
    uaT = np.ascontiguousarray(Ua_w.T).astype(BF16)
    waT = np.ascontiguousarray(Wa_w.T)
    wihT = np.ascontiguousarray(Wih_w.T)
    whhT = np.ascontiguousarray(Whh_w.T)
    va_rep = np.ascontiguousarray(
        np.broadcast_to(Va_w[0].astype(BF16).reshape(HC, P, 1), (HC, P, P)))
    qbias = (Wa_b + Ua_b)
    hbias = (Wih_b + Whh_b)

    x_bf = x_ref_encoded.astype(BF16)          # cast first (halves transpose traffic)
    in_maps = []
    for c in range(NCORES):
        rows = slice(c * BPC, (c + 1) * BPC)
        xT = np.ascontiguousarray(np.swapaxes(x_bf[rows], 1, 2))
        hpT = np.ascontiguousarray(h_prev[rows].T)
        xtT = np.ascontiguousarray(x_t[rows, 0, :].T)
        in_maps.append({
            "xT": xT, "uaT": uaT, "waT": waT, "va_rep": va_rep,
            "qbias": qbias, "hpT": hpT, "xtT": xtT,
            "wihT": wihT, "whhT": whhT, "hbias": hbias,
        })
    return in_maps


def kernel(x_t, x_ref_encoded, h_prev, Wa_w, Wa_b, Ua_w, Ua_b, Va_w, Va_b,
           Wih_w, Wih_b, Whh_w, Whh_b, _trace=False, _tmpdir=None):
    if "nc" not in _cache:
        _cache["nc"] = _build()
    nc = _cache["nc"]

    in_maps = _prep_host_inputs(
        np.asarray(x_t), np.asarray(x_ref_encoded), np.asarray(h_prev),
        np.asarray(Wa_w), np.asarray(Wa_b), np.asarray(Ua_w), np.asarray(Ua_b),
        np.asarray(Va_w), np.asarray(Va_b), np.asarray(Wih_w), np.asarray(Wih_b),
        np.asarray(Whh_w), np.asarray(Whh_b))

    res = run_bass_kernel_spmd(nc, in_maps, core_ids=list(range(NCORES)),
                               trace=_trace, tmpdir=_tmpdir)
    _cache["last_result"] = res

    h_next = np.concatenate([res.results[c]["h_next"] for c in range(NCORES)], axis=0)
    context = np.concatenate([res.results[c]["context"] for c in range(NCORES)], axis=0)
    return (h_next, context)

